# revision 22
# baseline (speedup 1.0000x reference)
"""Trainium2 Bass kernel for nn_MultiHeadAttention_85229331022244.

Computation (per batch b):
  xh = x.reshape(B,T,64,16); q/k/v = per-head 64x64 projections of xh
  q,k: interleaved RoPE over the FULL 1024-dim feature axis
  scores = q @ k.T / sqrt(1024)  (single attention map over full D)
  causal softmax; y = attn @ v

Sharding: core i -> batch i//2, q-block parity i%2 (even/odd 128-row q-blocks
interleaved between the two cores of a batch).  Every core runs an identical
program; parity differences are carried purely in DATA (a per-core key-block
permutation + 6 multiplicative mask tiles).

Dataflow (S-transposed flash):
  - heads reordered even-first and paired so projections are 8 block-diagonal
    128x128 matmuls producing K^T/Q^T in [feature, token] layout (as baseline).
  - scores computed TRANSPOSED: S^T[key, q] tiles [128, 256], so exp(S^T) is
    directly the lhsT of the attn@V matmuls -- no P transposes.
  - softmax row sums via N=1 matmuls against a ones vector.
  - causal masking via 6 multiplicative [128,128] masks (per-core data).
  - Q projection inputs are column slots of the SAME x stripes used for K/V
    (per-core key permutation puts each core's q-blocks at slots 1,3).
"""

import math
from contextlib import ExitStack

import numpy as np
import ml_dtypes

import concourse.bass as bass
import concourse.mybir as mybir
import concourse.tile as tile
from concourse import bacc
from concourse.bass import ts, ds

BF16 = ml_dtypes.bfloat16

D_MODEL = 1024
N_HEADS = 16
HEAD_D = 64
ROPE_BASE = 10000.0
GAMMA = 1.0 / math.sqrt(D_MODEL)
T = 4096
NSTR = T // 512  # 8 key stripes / q groups per core

HEAD_PAIRS = [(0, 2), (4, 6), (8, 10), (12, 14), (1, 3), (5, 7), (9, 11), (13, 15)]


def _feature_perm():
    perm = np.zeros(1024, dtype=np.int64)
    for c, (ha, hb) in enumerate(HEAD_PAIRS):
        for p in range(128):
            h = ha if p < 64 else hb
            perm[c * 128 + p] = (p % 64) * 16 + h
    return perm


PERM = _feature_perm()
INV_PERM = np.argsort(PERM)


def _block_weights(w):
    out = np.zeros((8, 128, 128), dtype=np.float32)
    for c, (ha, hb) in enumerate(HEAD_PAIRS):
        out[c, :64, :64] = w[:, :, ha]
        out[c, 64:, 64:] = w[:, :, hb]
    return out.astype(BF16)


def _freqs():
    p = np.arange(128)
    f = np.zeros((4, 128), dtype=np.float64)
    for c in range(4):
        fidx = (p % 64) * 8 + (2 * c + p // 64)
        f[c] = ROPE_BASE ** (-fidx / 512.0)
    return f


FREQS = _freqs()


def _kcols(parity):
    order = []
    for s in range(NSTR):
        if parity == 0:
            order += [4 * s + 1, 4 * s + 0, 4 * s + 3, 4 * s + 2]
        else:
            order += [4 * s + 0, 4 * s + 1, 4 * s + 2, 4 * s + 3]
    return np.concatenate([np.arange(128) + 128 * b for b in order])


def _msel(parity):
    r = np.arange(128)[:, None]
    c = np.arange(128)[None, :]
    tri = (r <= c).astype(np.float32)
    ones = np.ones((128, 128), np.float32)
    zeros = np.zeros((128, 128), np.float32)
    if parity == 0:
        m = [zeros, tri, zeros, zeros, zeros, tri]
    else:
        m = [ones, tri, zeros, zeros, ones, tri]
    return np.stack(m).astype(BF16)


def _rope_tables_neg(kc):
    """(-cos, -sin) tables at global t columns kc: each [4, 128, T] fp32."""
    t = np.asarray(kc, dtype=np.float64)
    ang = FREQS[:, :, None] * t[None, None, :]
    return -np.cos(ang), -np.sin(ang)


# ------------------------- device program -------------------------


def build_nc():
    dt = mybir.dt
    nc = bacc.Bacc("TRN2", target_bir_lowering=False)
    xS = nc.dram_tensor("xS", [NSTR, 128, 8, 512], dt.bfloat16, kind="ExternalInput")
    # angle-addition RoPE table inputs: per-block tables cos/sin(f * t_rel)
    # and per-(chunk,stripe) scalars cos/sin(f * 512 * s)
    bcsD = nc.dram_tensor("bcs", [4, 128, 2, 512], dt.bfloat16,
                          kind="ExternalInput")
    abscD = nc.dram_tensor("absc", [128, 64], dt.float32, kind="ExternalInput")
    w2q = nc.dram_tensor("w2q", [8, 128, 128], dt.bfloat16, kind="ExternalInput")
    w2k = nc.dram_tensor("w2k", [8, 128, 128], dt.bfloat16, kind="ExternalInput")
    w2v = nc.dram_tensor("w2v", [8, 128, 128], dt.bfloat16, kind="ExternalInput")
    mselD = nc.dram_tensor("msel", [6, 128, 128], dt.bfloat16, kind="ExternalInput")
    y = nc.dram_tensor("y", [2 * NSTR, 128, 1024], dt.bfloat16,
                       kind="ExternalOutput")

    with tile.TileContext(nc) as tc, ExitStack() as ctx:
        const = ctx.enter_context(tc.tile_pool(name="const", bufs=1))
        kv = ctx.enter_context(tc.tile_pool(name="kv", bufs=1))
        xpool = ctx.enter_context(tc.tile_pool(name="xpool", bufs=2))
        cspool = ctx.enter_context(tc.tile_pool(name="cspool", bufs=1))
        tgpool = ctx.enter_context(tc.tile_pool(name="tgpool", bufs=2))
        qpool = ctx.enter_context(tc.tile_pool(name="qpool", bufs=1))
        qcs = ctx.enter_context(tc.tile_pool(name="qcs", bufs=1))
        rtmp = ctx.enter_context(tc.tile_pool(name="rtmp", bufs=2))
        ptpool = ctx.enter_context(tc.tile_pool(name="ptpool", bufs=1))
        ypool = ctx.enter_context(tc.tile_pool(name="ypool", bufs=1))
        lpool = ctx.enter_context(tc.tile_pool(name="lpool", bufs=2))
        psS = ctx.enter_context(tc.tile_pool(name="psS", bufs=2, space="PSUM"))
        psY = ctx.enter_context(tc.tile_pool(name="psY", bufs=1, space="PSUM"))
        psL = ctx.enter_context(tc.tile_pool(name="psL", bufs=1, space="PSUM"))
        psP = ctx.enter_context(tc.tile_pool(name="psP", bufs=1, space="PSUM"))
        psV = ctx.enter_context(tc.tile_pool(name="psV", bufs=1, space="PSUM"))

        # ---- constants ----
        ones = const.tile([128, 1], dt.bfloat16, tag="ones", name="ones")
        nc.gpsimd.memset(ones[:], 1.0)
        bcs = []
        for c in range(4):
            bt = const.tile([128, 2, 512], dt.bfloat16, tag=f"bcs{c}",
                            name=f"bcs{c}")
            nc.sync.dma_start(bt[:], bcsD[c])
            bcs.append(bt)
        absc = const.tile([128, 64], dt.float32, tag="absc", name="absc")
        nc.sync.dma_start(absc[:], abscD[:, :])
        wq_sb, wk_sb, wv_sb = [], [], []
        for c in range(8):
            for nm, dram, lst in (("wq", w2q, wq_sb), ("wk", w2k, wk_sb),
                                  ("wv", w2v, wv_sb)):
                wt = const.tile([128, 128], dt.bfloat16, tag=f"{nm}{c}",
                                name=f"{nm}{c}")
                nc.sync.dma_start(wt[:], dram[c])
                lst.append(wt)
        msel = []
        for i in range(6):
            mt = const.tile([128, 128], dt.bfloat16, tag=f"msel{i}",
                            name=f"msel{i}")
            nc.sync.dma_start(mt[:], mselD[i])
            msel.append(mt)

        # resident K^T per (chunk, stripe) and V per local key block
        KT = {}
        for s in range(NSTR):
            for c in range(8):
                KT[(c, s)] = kv.tile([128, 512], dt.bfloat16, tag=f"kt{c}_{s}",
                                     name=f"kt{c}_{s}")
        V = [kv.tile([128, 1024], dt.bfloat16, tag=f"v{kb}", name=f"v{kb}")
             for kb in range(4 * NSTR)]

        def rope6(out_e, out_o, ke, ko, cos, sin, w):
            """out_e = ke*cos - ko*sin ; out_o = ke*sin + ko*cos (width w)."""
            ta = rtmp.tile([128, 512], dt.bfloat16, tag="ta", name="ta")
            tb = rtmp.tile([128, 512], dt.bfloat16, tag="tb", name="tb")
            nc.vector.tensor_mul(ta[:, :w], ke[:, :w], cos)
            nc.vector.tensor_mul(tb[:, :w], ko[:, :w], sin)
            nc.vector.tensor_sub(out_e, ta[:, :w], tb[:, :w])
            ta2 = rtmp.tile([128, 512], dt.bfloat16, tag="ta", name="ta")
            tb2 = rtmp.tile([128, 512], dt.bfloat16, tag="tb", name="tb")
            nc.vector.tensor_mul(ta2[:, :w], ke[:, :w], sin)
            nc.vector.tensor_mul(tb2[:, :w], ko[:, :w], cos)
            nc.vector.tensor_add(out_o, ta2[:, :w], tb2[:, :w])

        QT = {}

        def emit_stripe(s):
            xt = xpool.tile([128, 8, 512], dt.bfloat16, tag="xt", name="xt")
            nc.sync.dma_start(xt[:], xS[s])
            # cos/sin tables for this stripe by angle addition:
            #   cos(a+b) = ca*cos_b - sa*sin_b ; sin(a+b) = sa*cos_b + ca*sin_b
            # a = f*512*s (per-partition scalars), b = f*t_rel (bcs tables)
            cs = cspool.tile([128, 4, 2, 512], dt.bfloat16, tag="cs", name="cs")
            for cp in range(4):
                ca = absc[:, ds(cp * 16 + 2 * s, 1)]
                sa = absc[:, ds(cp * 16 + 2 * s + 1, 1)]
                t1 = tgpool.tile([128, 512], dt.bfloat16, tag="tg1", name="tg1")
                t2 = tgpool.tile([128, 512], dt.bfloat16, tag="tg2", name="tg2")
                nc.vector.tensor_scalar_mul(t1[:], bcs[cp][:, 0, :], ca)
                nc.vector.tensor_scalar_mul(t2[:], bcs[cp][:, 1, :], sa)
                nc.vector.tensor_sub(cs[:, cp, 0, :], t1[:], t2[:])
                t3 = tgpool.tile([128, 512], dt.bfloat16, tag="tg1", name="tg1")
                t4 = tgpool.tile([128, 512], dt.bfloat16, tag="tg2", name="tg2")
                nc.vector.tensor_scalar_mul(t3[:], bcs[cp][:, 0, :], sa)
                nc.vector.tensor_scalar_mul(t4[:], bcs[cp][:, 1, :], ca)
                nc.vector.tensor_add(cs[:, cp, 1, :], t3[:], t4[:])

            # K projection + RoPE (feature-on-partition layout)
            for cp in range(4):
                pe = psP.tile([128, 512], dt.float32, tag="pe", name="pe")
                po = psP.tile([128, 512], dt.float32, tag="po", name="po")
                nc.tensor.matmul(pe[:], lhsT=wk_sb[cp][:], rhs=xt[:, cp, :],
                                 start=True, stop=True)
                nc.tensor.matmul(po[:], lhsT=wk_sb[cp + 4][:],
                                 rhs=xt[:, cp + 4, :], start=True, stop=True)
                ke = rtmp.tile([128, 512], dt.bfloat16, tag="ke", name="ke")
                ko = rtmp.tile([128, 512], dt.bfloat16, tag="ko", name="ko")
                nc.scalar.copy(ke[:], pe[:])
                nc.scalar.copy(ko[:], po[:])
                rope6(KT[(cp, s)][:], KT[(cp + 4, s)][:], ke, ko,
                      cs[:, cp, 0, :], cs[:, cp, 1, :], 512)

            # V projection per local key block (keys on partitions)
            for j in range(4):
                kb = 4 * s + j
                for half in range(2):
                    pv = psV.tile([128, 512], dt.float32, tag="pv", name="pv")
                    for cc in range(4):
                        c = 4 * half + cc
                        nc.tensor.matmul(
                            pv[:, ts(cc, 128)],
                            lhsT=xt[:, c, ds(128 * j, 128)], rhs=wv_sb[c][:],
                            start=True, stop=True)
                    if half == 0:
                        nc.scalar.copy(V[kb][:, ds(0, 512)], pv[:])
                    else:
                        nc.vector.tensor_copy(V[kb][:, ds(512, 512)], pv[:])

            # Q tables (columns at slots 1,3 of this stripe) + Q proj + RoPE
            for cp in range(4):
                qc = qcs.tile([128, 256], dt.bfloat16, tag=f"qc{cp}",
                              name=f"qc{cp}")
                qs = qcs.tile([128, 256], dt.bfloat16, tag=f"qs{cp}",
                              name=f"qs{cp}")
                nc.vector.tensor_copy(qc[:, 0:128], cs[:, cp, 0, ds(128, 128)])
                nc.vector.tensor_copy(qc[:, 128:256], cs[:, cp, 0, ds(384, 128)])
                nc.vector.tensor_copy(qs[:, 0:128], cs[:, cp, 1, ds(128, 128)])
                nc.vector.tensor_copy(qs[:, 128:256], cs[:, cp, 1, ds(384, 128)])

                pe = psP.tile([128, 512], dt.float32, tag="pe", name="pe")
                po = psP.tile([128, 512], dt.float32, tag="po", name="po")
                for sl, off in ((0, 128), (1, 384)):
                    nc.tensor.matmul(pe[:, ts(sl, 128)], lhsT=wq_sb[cp][:],
                                     rhs=xt[:, cp, ds(off, 128)],
                                     start=True, stop=True)
                    nc.tensor.matmul(po[:, ts(sl, 128)], lhsT=wq_sb[cp + 4][:],
                                     rhs=xt[:, cp + 4, ds(off, 128)],
                                     start=True, stop=True)
                ke = rtmp.tile([128, 512], dt.bfloat16, tag="ke", name="ke")
                ko = rtmp.tile([128, 512], dt.bfloat16, tag="ko", name="ko")
                nc.scalar.copy(ke[:, :256], pe[:, :256])
                nc.scalar.copy(ko[:, :256], po[:, :256])
                qte = qpool.tile([128, 256], dt.bfloat16, tag=f"qt{cp}",
                                 name=f"qt{cp}")
                qto = qpool.tile([128, 256], dt.bfloat16, tag=f"qt{cp + 4}",
                                 name=f"qt{cp + 4}")
                rope6(qte[:], qto[:], ke, ko, qc[:], qs[:], 256)
                QT[cp] = qte
                QT[cp + 4] = qto

        def emit_q_group(g):
            nkb = 4 * g + 4
            pts = []
            for kb in range(nkb):
                S = psS.tile([128, 256], dt.float32, tag="S", name="S")
                for c in range(8):
                    nc.tensor.matmul(
                        S[:],
                        lhsT=KT[(c, kb // 4)][:, ts(kb % 4, 128)],
                        rhs=QT[c][:],
                        start=(c == 0), stop=(c == 7))
                pt = ptpool.tile([128, 256], dt.bfloat16, tag=f"pt{kb}",
                                 name=f"pt{kb}")
                nc.scalar.activation(pt[:], S[:],
                                     mybir.ActivationFunctionType.Exp,
                                     scale=GAMMA)
                pts.append(pt)
            # causal masking multiplies (last stripe's 4 blocks)
            for j in range(4):
                kb = 4 * g + j
                nc.vector.tensor_mul(pts[kb][:, 0:128], pts[kb][:, 0:128],
                                     msel[j][:])
            for jj, j in enumerate((2, 3)):
                kb = 4 * g + j
                nc.vector.tensor_mul(pts[kb][:, 128:256], pts[kb][:, 128:256],
                                     msel[4 + jj][:])
            # attn @ V + row sums, per owned q block m
            L = psL.tile([128, 2], dt.float32, tag="L", name="L")
            for m in range(2):
                Y = psY.tile([128, 1024], dt.float32, tag="Y", name="Y")
                for kb in range(nkb):
                    lhs = pts[kb][:, ds(128 * m, 128)]
                    nc.tensor.matmul(Y[:, 0:512], lhsT=lhs, rhs=V[kb][:, 0:512],
                                     start=(kb == 0), stop=(kb == nkb - 1))
                    nc.tensor.matmul(Y[:, 512:1024], lhsT=lhs,
                                     rhs=V[kb][:, 512:1024],
                                     start=(kb == 0), stop=(kb == nkb - 1))
                    nc.tensor.matmul(L[:, ds(m, 1)], lhsT=lhs, rhs=ones[:],
                                     start=(kb == 0), stop=(kb == nkb - 1))
                linv = lpool.tile([128, 1], dt.float32, tag="li", name="li")
                nc.vector.reciprocal(linv[:], L[:, ds(m, 1)])
                y_sb = ypool.tile([128, 1024], dt.bfloat16, tag="y", name="y")
                nc.vector.tensor_scalar_mul(y_sb[:], Y[:], linv[:])
                nc.sync.dma_start(y[2 * g + m], y_sb[:])

        for s in range(NSTR):
            emit_stripe(s)
            emit_q_group(s)

    nc.compile()
    return nc


# ------------------------- host side -------------------------


def _bcs(parity):
    """[4, 128, 2, 512] bf16: cos/sin(f * t_rel) in this core's block order."""
    rel = [1, 0, 3, 2] if parity == 0 else [0, 1, 2, 3]
    t_rel = np.concatenate([128 * r + np.arange(128) for r in rel])
    ang = FREQS[:, :, None] * t_rel[None, None, :]
    return np.stack([np.cos(ang), np.sin(ang)], axis=2).astype(BF16)


def _absc():
    """[128, 64] fp32: cos/sin(f * 512 * s) at col c*16 + 2*s (+1 for sin)."""
    out = np.zeros((128, 64), np.float64)
    for c in range(4):
        for s in range(NSTR):
            a = FREQS[c] * 512.0 * s
            out[:, c * 16 + 2 * s] = np.cos(a)
            out[:, c * 16 + 2 * s + 1] = np.sin(a)
    return out.astype(np.float32)


ABSC = _absc()


def prep_core_inputs(xb, w2q, w2k, w2v, parity):
    """Inputs for one core: batch slice xb (T, 1024) fp32, parity 0/1."""
    kc = _kcols(parity)
    xpT = np.ascontiguousarray(xb.T[PERM]).reshape(8, 128, T)
    xperm = xpT[:, :, kc]
    xS = np.ascontiguousarray(
        xperm.reshape(8, 128, NSTR, 512).transpose(2, 1, 0, 3)).astype(BF16)
    return {
        "xS": xS,
        "bcs": _bcs(parity),
        "absc": ABSC,
        "w2q": w2q,
        "w2k": w2k,
        "w2v": w2v,
        "msel": _msel(parity),
    }


_NC_CACHE = {}
last_in_maps = None
last_nc = None


def kernel(x, w_q, w_k, w_v):
    global last_in_maps, last_nc
    from concourse.bass_utils import run_bass_kernel_spmd

    B, Tx, D = x.shape
    assert (B, Tx, D) == (4, 4096, 1024)
    x = np.asarray(x, dtype=np.float32)
    w2q = _block_weights(np.asarray(w_q, dtype=np.float32))
    w2k = _block_weights(np.asarray(w_k, dtype=np.float32))
    w2v = _block_weights(np.asarray(w_v, dtype=np.float32))

    in_maps = []
    for core in range(8):
        b, parity = core // 2, core % 2
        in_maps.append(prep_core_inputs(x[b], w2q, w2k, w2v, parity))
    last_in_maps = in_maps

    if "nc" not in _NC_CACHE:
        _NC_CACHE["nc"] = build_nc()
    nc = _NC_CACHE["nc"]
    last_nc = nc

    res = run_bass_kernel_spmd(nc, in_maps, core_ids=list(range(8)))
    out = np.zeros((B, Tx, D), dtype=np.float32)
    for core in range(8):
        b, parity = core // 2, core % 2
        yk = res.results[core]["y"].astype(np.float32)  # [16, 128, 1024]
        for g in range(NSTR):
            for m in range(2):
                G = 4 * g + 2 * m + parity
                out[b, 128 * G:128 * (G + 1), :] = yk[2 * g + m][:, INV_PERM]
    return out
